# revision 26
# baseline (speedup 1.0000x reference)
"""Trainium2 Bass kernel for: out = (x @ wsums.sum(0)) * (1.5 * 0.5).

x: [1024, 8192] f32, wsums: [32, 8192] f32 -> out: [1024, 1] f32.

Sharding across 8 NeuronCores: 8-way along the contraction dim k
(8192 -> 1024 per core).  Each core reads a 4MB x column-shard plus its
128KB wsums k-slice, computes partial dot products for ALL 1024 rows over
its k-slice, and the host sums the 8 per-core partials (the unshard step
for a contraction-sharded dim).

Final design (v1 f32/HWDGE baseline: 29.4-30.2us; this version samples
27.0-29.2us, mean-across-cores ~27.2us; run-to-run spread is chip DVFS +
which cores hit the slow SDMA engine):
  - x and wsums are DMA'd through the SWDGE (gpsimd) path with an f32->bf16
    cast in the SDMA datapath.  HBM read bytes are unchanged (the memory
    bound) and traces show the same ~430GB/s aggregate read rate as HWDGE,
    but the tensor_tensor multiplies then run at the DVE 2x bf16 packed
    rate (measured 1.23us vs 2.29us per 1MB chunk).
  - All tiles are 128-partition: partial-partition DMAs (124-wide) skew
    the SWDGE descriptor->engine distribution badly (127/37/95 packets
    per engine vs ~74 even), collapsing DMA throughput.
  - Pool (GpSimd) is kept OFF the compute path: its tensor_tensor adds
    measured ~2.6x slower than the cost model, and any waiting compute on
    Pool's in-order stream also blocks later piece DMA triggers.
  - Single-block x DMAs: each completion semaphore needs only 8
    descriptors from the known-slow SDMA engines 7/15 (a ~2-3us straggle
    on 1-2 cores per run), so chunks become compute-ready sooner.
  - The per-element accumulation pass is the wall (~1 elem/cycle on DVE
    and ScalarE alike, dtype-independent: ACT 1.43us / DVE reduce 1.21us
    per 1024-wide block).  It is split arrival-aware: DVE reduces ride the
    gaps between its TTs (blk1, blk3), ScalarE takes the mid-stream
    blocks, and the two late blocks (7 and 0) are k-split so both engines
    accumulate the tail in parallel.
  - acc columns 0..7 are stored while block 0 computes; only a 2-column
    store + completion receipt trails the final 256-wide reduce.  The
    host folds the k-split partial pairs (it already sums 8 per-core
    partials, so extra partial columns are free).

Environment workarounds (this container's walrus build):
  - it encodes at most ONE semaphore wait per instruction ("Too many sync
    wait commands"), so compile_bir_kernel is wrapped with a BIR post-pass
    that moves excess waits onto preceding same-engine NoOp instructions;
  - it cannot encode bass_isa raw-ISA ops (tensor_tensor_reduce,
    partition_all_reduce, ... -> "ISA wrong length"), so only classic
    mybir ops are used.
"""

import json

import numpy as np

import concourse.bass as bass
import concourse.bass2jax as bass2jax
import concourse.bass_utils as bass_utils
import concourse.mybir as mybir
from concourse.tile import TileContext

SCALE = 1.5 * 0.5
B, K, G = 1024, 8192, 32
N_CORES = 8
KSHARD = 8                  # cores along k
BSHARD = N_CORES // KSHARD  # cores along batch
KB = K // KSHARD            # per-core k width
BB = B // BSHARD            # per-core rows
P = 128
NBLK = BB // P              # row-blocks per core
# acc columns: blk1..6 -> 0..5, blk7 k-lo/hi -> 6/7, blk0 k-lo/hi -> 8/9
NCOL = NBLK + 2
F32 = mybir.dt.float32
BF16 = mybir.dt.bfloat16

# Set by test.py to profile; results stashed in LAST_RESULTS.
TRACE = False
TRACE_KWARGS = {}
LAST_RESULTS = None

_built = None

# Accumulating engine per piece.  DVE ("vector") reduces are placed where
# DVE has arrival gaps between its TTs; ScalarE takes the rest.  The late
# pieces (blk7, blk0) are k-halves so their accumulations spread across
# both engines at the tail.
ACCUM_PLAN = {
    "blk1": "vector",
    "blk2": "scalar",
    "blk3": "vector",
    "blk4": "scalar",
    "blk5": "scalar",
    "blk6": "scalar",
    "b7lo": "scalar",
    "b7hi": "vector",
    "b0lo": "scalar",
    "b0hi": "vector",
}

# ---------------------------------------------------------------------------
# Workaround: this container's walrus encodes at most 1 sync wait per
# instruction.  Split longer on_wait lists onto preceding same-engine NoOps.
MAX_WAITS = 1
_orig_compile_bir_kernel = bass_utils.compile_bir_kernel


def _split_waits_in_bir(bir: dict) -> int:
    counter = [0]

    def fix_blocks(blocks):
        for bb in blocks:
            out = []
            for ins in bb.get("instructions", []):
                si = ins.get("sync_info")
                ow = (si or {}).get("on_wait") or []
                if len(ow) > MAX_WAITS:
                    extra, keep = ow[:-MAX_WAITS], ow[-MAX_WAITS:]
                    for i in range(0, len(extra), MAX_WAITS):
                        counter[0] += 1
                        out.append({
                            "name": f"I-waitsplit-{counter[0]}",
                            "engine": ins["engine"],
                            "opcode": "NoOp",
                            "ins": [],
                            "outs": [],
                            "debug": ins.get("debug", 0),
                            "sync_info": {
                                "on_update": [],
                                "on_wait": extra[i : i + MAX_WAITS],
                            },
                        })
                    si["on_wait"] = keep
                out.append(ins)
            bb["instructions"] = out
            if bb.get("blocks"):
                fix_blocks(bb["blocks"])

    for fn in bir["functions"]:
        fix_blocks(fn["blocks"])
    return counter[0]


def _patched_compile_bir_kernel(bir_json, tmpdir, neff_name="file.neff"):
    if isinstance(bir_json, str):
        bir_json = bir_json.encode()
    bir = json.loads(bir_json)
    _split_waits_in_bir(bir)
    return _orig_compile_bir_kernel(json.dumps(bir).encode(), tmpdir, neff_name)


bass_utils.compile_bir_kernel = _patched_compile_bir_kernel
bass2jax.compile_bir_kernel = _patched_compile_bir_kernel

# ---------------------------------------------------------------------------
# The walrus postamble zeroes all 256 semaphores (51 per engine) after the
# body, ~6.7us of the measured window on every execution.  The kernel only
# allocates ~25 sems, so cap the semaphore space: the wipe shrinks
# proportionally and still covers every allocatable sem, keeping
# re-execution correct by construction.
MAX_SEM_NUM = None  # 64 made the kernel produce NaNs (sem renumbering race)
_orig_get_walrus_args = bass_utils.get_walrus_args


def _patched_get_walrus_args(*a, **k):
    extra = [] if MAX_SEM_NUM is None else [f"--max-sem-num={MAX_SEM_NUM}"]
    return [*_orig_get_walrus_args(*a, **k), *extra]


bass_utils.get_walrus_args = _patched_get_walrus_args


# ---------------------------------------------------------------------------
# Overlapped TileContext exit.  The stock exit serializes: drain(+DMA-sem
# waits) -> all-engine barrier -> sem clears -> barrier, so every engine's
# walrus postamble starts only after the out-DMA's completion receipt.
# Instead: Sync drains with the global-clock + DMA-completion waits, then
# incs a handoff semaphore; GpSimd and Vector wait for the handoff before
# entering their postambles; Tensor and Scalar get no tail instructions at
# all.  The explicit Tile sem clears are dropped: the walrus postamble wipes
# all 256 semaphores every execution, which keeps re-execution correct.
import concourse.tile as tile_mod
from concourse.tile import TileContext as _TC


def _overlap_drain_and_barrier(self, tick_clock, wait_clock):
    nc = self.nc
    drain_inst = nc.sync.drain()
    wait_clock.add_sem_waits(
        drain_inst.ins,
        tile_mod.ScopedClock({None: tick_clock.global_clock}),
    )
    done = nc.alloc_semaphore("tail_dma_done")
    # Tensor/Scalar's early postambles may zero this sem, but they run long
    # before Sync's drain increments it, so a clear-then-inc is harmless.
    drain_inst.then_inc(done, 1)
    nc.gpsimd.wait_ge(done, 1)
    nc.vector.wait_ge(done, 1)
    popped = nc._tile_sem_poison_stack.pop()
    assert popped is self._sem_poison


_TC._drain_and_barrier = _overlap_drain_and_barrier
# ---------------------------------------------------------------------------


def _build():
    # Bass.__init__ ends with an all-engine barrier ordering its const-AP
    # memsets against the body.  This kernel never reads those const APs,
    # and the NRT start barrier already aligns the engines at execution
    # start, so skip it.
    _orig_aeb = bass.Bass.all_engine_barrier
    bass.Bass.all_engine_barrier = lambda self, **kw: None
    try:
        nc = bass.Bass("TRN2")
    finally:
        bass.Bass.all_engine_barrier = _orig_aeb
    x_sh = nc.dram_tensor("x_shard", (BB, KB), F32, kind="ExternalInput")
    w_sh = nc.dram_tensor("wsums_shard", (G, KB), F32, kind="ExternalInput")
    out = nc.dram_tensor("out_acc", (P, NCOL), F32, kind="ExternalOutput")

    def accumulate(name, yt, acc_ap):
        if ACCUM_PLAN[name] == "scalar":
            nc.scalar.activation(
                yt, yt, mybir.ActivationFunctionType.Copy, accum_out=acc_ap
            )
        else:
            nc.vector.tensor_reduce(
                acc_ap, yt, axis=mybir.AxisListType.X, op=mybir.AluOpType.add
            )

    with TileContext(nc) as tc:
        with (
            tc.tile_pool(name="const", bufs=1) as cpool,
            tc.tile_pool(name="psum", bufs=1, space="PSUM") as ppool,
        ):
            # wsums slice, cast f32->bf16 in the SDMA datapath (SWDGE).
            # Its descriptor emission is interleaved after the first x
            # piece's so the x stream starts ~0.6us earlier; wp is still
            # ready (~13.3us) before the first chunk's multiply needs it.
            ws = cpool.tile([G, KB], BF16)

            # Pieces in DMA/processing order: (names, blocks, k0, kw, cols).
            # Single-block DMAs: each completion semaphore needs only 8
            # descriptors from the slow SDMA engine (15) instead of 16, so
            # arrivals run ~1.3us earlier on straggler cores.  A piece's
            # accumulation splits evenly over its names (for blk7 that
            # means two k-halves, one per engine, at the tail).
            pieces = [
                (("blk1",), (1,), 0, KB, (0,)),
                (("blk2",), (2,), 0, KB, (1,)),
                (("blk3",), (3,), 0, KB, (2,)),
                (("blk4",), (4,), 0, KB, (3,)),
                (("blk5",), (5,), 0, KB, (4,)),
                (("blk6",), (6,), 0, KB, (5,)),
                (("b7lo", "b7hi"), (7,), 0, KB, (6, 7)),
                (("b0lo", "b0hi"), (0,), 0, KB, (8, 9)),
            ]

            # Trigger every piece's DMA up-front on GpSimd so the SWDGE
            # emission (~0.7-1.4us per piece, serial on Q7) finishes as
            # early as possible; all x tiles are SBUF-resident.
            xts = []
            for pi, (names, blocks, k0, kw, cols) in enumerate(pieces):
                nrb = len(blocks)
                rb0 = blocks[0]
                assert blocks == tuple(range(rb0, rb0 + nrb))
                xt = cpool.tile(
                    [P, nrb * kw], BF16, tag=f"xt{pi}", name=f"xt{pi}"
                )
                # src[p, a, k] = x_shard[(rb0 + a) * P + p, k0 + k]
                src = bass.AP(
                    x_sh,
                    rb0 * P * KB + k0,
                    [[KB, P], [P * KB, nrb], [1, kw]],
                )
                nc.gpsimd.dma_start(out=xt, in_=src)
                xts.append(xt)
                if pi == 0:
                    nc.gpsimd.dma_start(out=ws, in_=w_sh.ap())
                elif pi == 1:
                    # Stationary = SCALE (exact in bf16): folds the output
                    # scale into the broadcast matmul.  Emitted between x
                    # DMA triggers; the matmul needs it ~2.5us later.
                    ones = cpool.tile([G, P], BF16, name="ones")
                    nc.gpsimd.memset(ones, SCALE)

            # wp_ps[m, n] = sum_g ones[g, m] * ws[g, n] = SCALE*w_total[n]
            # on every partition m.  N<=512 per matmul (one PSUM bank each).
            wp_ps = ppool.tile([P, KB], F32)
            for j in range(KB // 512):
                nc.tensor.matmul(
                    wp_ps[:, j * 512 : (j + 1) * 512],
                    ones,
                    ws[:, j * 512 : (j + 1) * 512],
                    start=True,
                    stop=True,
                )
            # PSUM f32 -> SBUF bf16 so the tensor_tensor runs in the DVE
            # 2x packed mode (PSUM operands force 1x).  On ScalarE: it is
            # idle until the first accumulation (~4us later) and sits
            # closer to PSUM, while DVE's budget is the tighter one.
            wp = cpool.tile([P, KB], BF16)
            nc.scalar.activation(wp, wp_ps, mybir.ActivationFunctionType.Copy)

            acc = cpool.tile([P, NCOL], F32)

            for pi, (names, blocks, k0, kw, cols) in enumerate(pieces):
                nrb = len(blocks)
                xt = xts[pi]
                yt = cpool.tile(
                    [P, nrb * kw], BF16, tag=f"yt{pi}", name=f"yt{pi}"
                )
                if nrb == 1:
                    nc.vector.tensor_tensor(
                        yt, xt, wp[:, k0 : k0 + kw], op=mybir.AluOpType.mult
                    )
                else:
                    # One fused multiply over nrb row-blocks; wp is repeated
                    # along a stride-0 middle dim.
                    x3 = xt[:].rearrange("p (a k) -> p a k", a=nrb)
                    y3 = yt[:].rearrange("p (a k) -> p a k", a=nrb)
                    wb = (
                        wp[:, k0 : k0 + kw]
                        .unsqueeze(1)
                        .broadcast_to([P, nrb, kw])
                    )
                    nc.vector.tensor_tensor(y3, x3, wb, op=mybir.AluOpType.mult)
                seg = (nrb * kw) // len(names)
                for a, name in enumerate(names):
                    accumulate(
                        name,
                        yt[:, a * seg : (a + 1) * seg],
                        acc[:, cols[a] : cols[a] + 1],
                    )
                if pi == 6:
                    # Columns 0..7 (blk1-7) are final once the blk7 halves
                    # are accumulated: store them while block 0 is still
                    # in flight; only the 2-column block-0 store plus its
                    # completion receipt trail the final reduce.
                    nc.sync.dma_start(
                        out=out.ap()[:, 0 : NCOL - 2], in_=acc[:, 0 : NCOL - 2]
                    )
            nc.sync.dma_start(
                out=out.ap()[:, NCOL - 2 : NCOL], in_=acc[:, NCOL - 2 : NCOL]
            )
    return nc


def kernel(x: np.ndarray, wsums: np.ndarray) -> np.ndarray:
    global _built, LAST_RESULTS
    if _built is None:
        _built = _build()
    nc = _built

    x = np.ascontiguousarray(np.asarray(x, dtype=np.float32))
    wsums = np.ascontiguousarray(np.asarray(wsums, dtype=np.float32))

    in_maps = []
    for c in range(N_CORES):
        bb_i, kb_i = divmod(c, KSHARD)
        xs = np.ascontiguousarray(
            x[bb_i * BB : (bb_i + 1) * BB, kb_i * KB : (kb_i + 1) * KB]
        )
        wsl = np.ascontiguousarray(wsums[:, kb_i * KB : (kb_i + 1) * KB])
        in_maps.append({"x_shard": xs, "wsums_shard": wsl})

    res = bass_utils.run_bass_kernel_spmd(
        nc,
        in_maps,
        core_ids=list(range(N_CORES)),
        trace=TRACE,
        **TRACE_KWARGS,
    )
    LAST_RESULTS = res

    parts = []
    for bb_i in range(BSHARD):
        tot = None
        for kb_i in range(KSHARD):
            acc = res.results[bb_i * KSHARD + kb_i]["out_acc"]  # [P, NCOL]
            # cols 0..5 = blk1..6; blk7 = col6+col7; blk0 = col8+col9.
            blk = np.concatenate(
                [
                    (acc[:, 8] + acc[:, 9])[:, None],
                    acc[:, 0:6],
                    (acc[:, 6] + acc[:, 7])[:, None],
                ],
                axis=1,
            )  # [P, NBLK] in block order 0..7
            vec = blk.T.reshape(BB)  # row 128*j + p  <-  blk[p, j]
            tot = vec if tot is None else tot + vec
        parts.append(tot)
    return np.concatenate(parts).astype(np.float32)[:, None]
